# revision 24
# baseline (speedup 1.0000x reference)
"""Trainium2 Bass kernel for CustomTripletMarginLoss (retrieval_knn).

Strategy (matches the sharding hint): shard the 8192 anchors across the 8
NeuronCores (1024 anchors each). Anchors are packed [128 partitions x 8
tiles] per core.

v2: the negated squared coordinate distances
    nd2[p, j] = -((cx[j] - ax[p])^2 + (cy[j] - ay[p])^2)
are produced by the (otherwise idle) TensorEngine via the quadratic
expansion as a K=4 matmul:
    nd2 = 2*ax*cx + 2*ay*cy - an2*1 - 1*cn2
with lhsT = [2ax; 2ay; -an2; -1] (stationary, [4, 128] per tile) and
rhs = [cx; cy; 1; cn2] (moving, [4, 8192]). an2 is host-gathered from the
same cn2 array so the self column sums to exactly 0.0 under fp32 PSUM
accumulation; self therefore stays the row max of nd2 (slot 0 of max8) and
no explicit self-masking is needed. ACT evicts PSUM quarters to SBUF; DVE
does the same one-sweep mining as v1 (row-min, max8, find_index8) but no
longer spends passes producing nd2 (which was the v1 bottleneck: DVE was
85% busy with 4 full sweeps per tile).

Mining works in negated squared-distance space (sqrt is monotone;
d < 5 <=> d2 < 25). A row is valid iff min nd2 <= -25 (a negative exists)
and the nearest non-self neighbor (max8 slot 1) has nd2 > -25 (a positive
exists). Embedding rows for pos/neg are gathered with indirect DMA, the
triplet loss terms are computed on-chip, and each core writes per-anchor
masked losses + valid flags; the host sums and divides.

v3: the neg side (argmax of distance) is exact over the convex hull of
the 2D coordinates: ||a - x|| is convex in x, so its max over the point
set is attained at a hull vertex. The host computes the hull (10 vertices
for this data; padded to 16, sorted ascending so find_index8's
first-occurrence matches jnp.argmax tie-breaking), and the device mines
neg over just those 16 candidates with exact ACT (q - a)^2 arithmetic.
This removes the full-row TENSOR_REDUCE(min) sweep - 1 of the 3 DVE
sweeps per tile - and the neg embedding is gathered from a host-staged
16-row candidate table.
"""

import numpy as np

import concourse.bacc as bacc
import concourse.bass as bass
import concourse.mybir as mybir
from concourse.bass import IndirectOffsetOnAxis
from concourse.bass_utils import run_bass_kernel_spmd
from concourse.tile import TileContext

N = 8192          # samples / anchors
D = 512           # embedding dim
NCORES = 8
P = 128           # partitions
PA = N // NCORES  # anchors per core
T = PA // P       # row-tiles per core
MARGIN = 0.1
NTHRESH = -25.0   # negated squared mining threshold: (MARGIN*100/2)^2 = 25
QC = 2048         # psum quarter-tile columns (4 banks)
MM = 512          # max moving free dim per matmul
H = 16            # neg-side hull candidate slots (adaptive: grown in
                  # kernel() if the hull is larger; >= 8 for find_index8)

F32 = mybir.dt.float32
I32 = mybir.dt.int32
U32 = mybir.dt.uint32

# test.py hooks: set TRACE=True before calling kernel() to capture an NTFF
# profile; the raw BassKernelResults land in LAST_RESULTS.
TRACE = False
LAST_RESULTS = None


def _build_program() -> bass.Bass:
    Act = mybir.ActivationFunctionType
    Alu = mybir.AluOpType

    nc = bacc.Bacc()
    emb = nc.declare_dram_parameter("emb", [N, D], F32, isOutput=False)
    rhs = nc.declare_dram_parameter("rhs", [4, N], F32, isOutput=False)
    lt = nc.declare_dram_parameter("lt", [4, PA], F32, isOutput=False)
    nax = nc.declare_dram_parameter("nax", [P, T], F32, isOutput=False)
    nay = nc.declare_dram_parameter("nay", [P, T], F32, isOutput=False)
    # anchor embeddings, host pre-gathered (anchor_idx is host-known), laid
    # out so tile t is the column block [:, t*D:(t+1)*D]
    ae = nc.declare_dram_parameter("ae", [P, T * D], F32, isOutput=False)
    # hull candidates: coords broadcast to all partitions + embedding table
    qcx = nc.declare_dram_parameter("qcx", [P, H], F32, isOutput=False)
    qcy = nc.declare_dram_parameter("qcy", [P, H], F32, isOutput=False)
    qemb = nc.declare_dram_parameter("qemb", [H, D], F32, isOutput=False)
    o_tlm = nc.declare_dram_parameter("tlm", [P, T], F32, isOutput=True)
    o_vld = nc.declare_dram_parameter("vld", [P, T], F32, isOutput=True)
    # debug: per tile t, col t*8+0 = pos_idx, col t*8+7 = neg_idx
    o_idx = nc.declare_dram_parameter("idx", [P, T * 8], U32, isOutput=True)

    with TileContext(nc) as tc:
        with (
            tc.tile_pool(name="const", bufs=1) as pc,
            tc.tile_pool(name="nd", bufs=2) as pn,
            tc.tile_pool(name="psum", bufs=2, space="PSUM") as pp,
            tc.tile_pool(name="small", bufs=3) as ps,
            tc.tile_pool(name="embt", bufs=2) as pe,
        ):
            rhs_t = pc.tile_from(rhs[:], name="rhs_t")
            lt_t = pc.tile_from(lt[:], name="lt_t")
            nax_t = pc.tile_from(nax[:], name="nax_t")
            nay_t = pc.tile_from(nay[:], name="nay_t")
            qcx_t = pc.tile_from(qcx[:], name="qcx_t")
            qcy_t = pc.tile_from(qcy[:], name="qcy_t")
            zero_t = pc.tile([P, 1], F32, name="zero_t")
            margin_t = pc.tile([P, 1], F32, name="margin_t")
            nc.vector.memset(zero_t[:], 0.0)
            nc.vector.memset(margin_t[:], MARGIN)
            tlm_acc = pc.tile([P, T], F32, name="tlm_acc")
            vld_acc = pc.tile([P, T], F32, name="vld_acc")
            idx_acc = pc.tile([P, T * 8], U32, name="idx_acc")

            def phase2(st):
                # triplet-loss tail for a mined tile; DVE subs run after the
                # NEXT tile's mining start so the indirect gathers have landed
                t = st["t"]
                dp = pe.tile([P, D], F32, name="dp")
                dn = pe.tile([P, D], F32, name="dn")
                nc.vector.tensor_sub(dp, st["ae_g"], st["pe_g"])
                nc.vector.tensor_sub(dn, st["ae_g"], st["ne_g"])
                sqp = pe.tile([P, D], F32, name="sqp")
                sqn = pe.tile([P, D], F32, name="sqn")
                pd2 = ps.tile([P, 1], F32, name="pd2")
                nd2e = ps.tile([P, 1], F32, name="nd2e")
                nc.scalar.activation(sqp, dp, Act.Square, bias=zero_t[:],
                                     accum_out=pd2)
                nc.scalar.activation(sqn, dn, Act.Square, bias=zero_t[:],
                                     accum_out=nd2e)
                posd = ps.tile([P, 1], F32, name="posd")
                negd = ps.tile([P, 1], F32, name="negd")
                nc.scalar.activation(posd, pd2, Act.Sqrt, bias=zero_t[:])
                nc.scalar.activation(negd, nd2e, Act.Sqrt, bias=zero_t[:])
                pmn = ps.tile([P, 1], F32, name="pmn")
                nc.vector.tensor_sub(pmn, posd, negd)
                tl = ps.tile([P, 1], F32, name="tl")
                nc.scalar.activation(tl, pmn, Act.Relu, bias=margin_t[:],
                                     scale=1.0)
                v2 = ps.tile([P, 1], F32, name="v2")
                nc.vector.tensor_scalar(out=v2, in0=st["mr"][:, 1:2],
                                        scalar1=NTHRESH, scalar2=None,
                                        op0=Alu.is_gt)
                nc.vector.scalar_tensor_tensor(
                    out=vld_acc[:, t:t + 1], in0=st["mr"][:, 8:9],
                    scalar=NTHRESH, in1=v2, op0=Alu.is_le, op1=Alu.mult)
                nc.vector.tensor_mul(tlm_acc[:, t:t + 1], tl,
                                     vld_acc[:, t:t + 1])

            pending = None
            for t in range(T):
                nd2 = pn.tile([P, N], F32, name="nd2")
                for h in range(N // QC):
                    pt = pp.tile([P, QC], F32, name="pt")
                    for c in range(QC // MM):
                        j0 = h * QC + c * MM
                        nc.tensor.matmul(
                            pt[:, c * MM:(c + 1) * MM],
                            lt_t[:, t * P:(t + 1) * P],
                            rhs_t[:, j0:j0 + MM],
                            start=True, stop=True)
                    nc.scalar.activation(nd2[:, h * QC:(h + 1) * QC], pt[:],
                                         Act.Copy, bias=0.0, scale=1.0)

                # neg side: exact (q - a)^2 over the H hull candidates;
                # mr packs [max8 | cand-negmin] so mr[:, 1:2] (nn value) and
                # mr[:, 8:9] (global min of nd2, via the hull) feed the same
                # validity checks as before
                mr = ps.tile([P, 9], F32, name="mr")
                uq = ps.tile([P, H], F32, name="uq")
                vq = ps.tile([P, H], F32, name="vq")
                nq = ps.tile([P, H], F32, name="nq")
                nc.scalar.activation(uq, qcx_t[:], Act.Square,
                                     bias=nax_t[:, t:t + 1], scale=1.0)
                nc.scalar.activation(vq, qcy_t[:], Act.Square,
                                     bias=nay_t[:, t:t + 1], scale=1.0)
                nc.vector.scalar_tensor_tensor(
                    out=nq, in0=uq, scalar=-1.0, in1=vq,
                    op0=Alu.mult, op1=Alu.subtract)
                nc.vector.tensor_reduce(
                    out=mr[:, 8:9], in_=nq, axis=mybir.AxisListType.X,
                    op=Alu.min)
                cidx = ps.tile([P, 8], U32, name="cidx")
                nc.vector.max_index(out=cidx,
                                    in_max=mr[:, 8:9].broadcast_to([P, 8]),
                                    in_values=nq)
                ne_g = pe.tile([P, D], F32, name="ne_g")
                nc.gpsimd.indirect_dma_start(
                    out=ne_g, out_offset=None, in_=qemb[:],
                    in_offset=IndirectOffsetOnAxis(
                        ap=cidx[:, 0:1], axis=0))
                if pending is not None:
                    # previous tile's loss tail lands here, two DVE passes
                    # after its gathers were issued, so they have surely
                    # completed and the subs don't stall
                    phase2(pending)
                nc.vector.max(out=mr[:, 0:8], in_=nd2)
                # probes are max8's own output, so every probe is present in
                # the row; slot 1 is the nearest non-self neighbor (slot 0 is
                # the anchor itself at exactly 0.0)
                nc.vector.max_index(out=idx_acc[:, t * 8:(t + 1) * 8],
                                    in_max=mr[:, 0:8], in_values=nd2)

                ae_g = pe.tile([P, D], F32, name="ae_g")
                pe_g = pe.tile([P, D], F32, name="pe_g")
                nc.sync.dma_start(out=ae_g, in_=ae[:, t * D:(t + 1) * D])
                nc.gpsimd.indirect_dma_start(
                    out=pe_g, out_offset=None, in_=emb[:],
                    in_offset=IndirectOffsetOnAxis(
                        ap=idx_acc[:, t * 8 + 1:t * 8 + 2], axis=0))
                pending = {"t": t, "ae_g": ae_g, "pe_g": pe_g, "ne_g": ne_g,
                           "mr": mr}
            phase2(pending)

            nc.sync.dma_start(out=o_tlm[:], in_=tlm_acc[:])
            nc.sync.dma_start(out=o_vld[:], in_=vld_acc[:])
            nc.sync.dma_start(out=o_idx[:], in_=idx_acc[:])
    # Bacc.finalize runs compile() (event-semaphore legalization for
    # multi-wait instructions, register allocation, DCE). The PJRT run
    # path does not call it for prebuilt modules.
    nc.finalize()
    return nc


def _hull_indices(cx, cy):
    """Andrew monotone chain; returns sorted original indices of the hull
    vertices of the (cx, cy) point set."""
    pts = np.stack([cx, cy], axis=1).astype(np.float64)
    order = np.lexsort((pts[:, 1], pts[:, 0]))
    p = pts[order]

    def half(rng):
        st = []
        for i in rng:
            while len(st) >= 2:
                o, a = p[st[-2]], p[st[-1]]
                if (a[0] - o[0]) * (p[i][1] - o[1]) - \
                   (a[1] - o[1]) * (p[i][0] - o[0]) <= 0:
                    st.pop()
                else:
                    break
            st.append(i)
        return st

    lower = half(range(len(p)))
    upper = half(range(len(p) - 1, -1, -1))
    return np.unique(order[np.array(lower[:-1] + upper[:-1])])


def make_in_maps(embeddings, coordinates, anchor_idx):
    emb = np.ascontiguousarray(np.asarray(embeddings, dtype=np.float32))
    coord = np.asarray(coordinates, dtype=np.float32)
    ai = np.asarray(anchor_idx).astype(np.int64)
    cx, cy = coord[:, 0].copy(), coord[:, 1].copy()
    cn2 = cx * cx + cy * cy  # fp32; an2 below indexes this same array so
    # the self column of the matmul sums to exactly zero
    rhs = np.ascontiguousarray(
        np.stack([cx, cy, np.ones(N, np.float32), cn2]).astype(np.float32))
    global H
    hv = _hull_indices(cx, cy)
    if len(hv) > H:
        H = int(-(-len(hv) // 8) * 8)
    hv = np.concatenate([hv, np.full(H - len(hv), hv[0], dtype=hv.dtype)])
    qcx = np.ascontiguousarray(
        np.broadcast_to(cx[hv], (P, H)).astype(np.float32))
    qcy = np.ascontiguousarray(
        np.broadcast_to(cy[hv], (P, H)).astype(np.float32))
    qemb = np.ascontiguousarray(emb[hv])
    in_maps = []
    for k in range(NCORES):
        sl = ai[k * PA:(k + 1) * PA]
        ae_core = emb[sl].reshape(T, P, D).transpose(1, 0, 2).reshape(P, T * D)
        lt = np.ascontiguousarray(np.stack([
            2.0 * cx[sl], 2.0 * cy[sl], -cn2[sl],
            -np.ones(PA, np.float32)]).astype(np.float32))
        in_maps.append({
            "emb": emb,
            "rhs": rhs,
            "lt": lt,
            "nax": np.ascontiguousarray((-cx[sl]).reshape(T, P).T),
            "nay": np.ascontiguousarray((-cy[sl]).reshape(T, P).T),
            "ae": np.ascontiguousarray(ae_core),
            "qcx": qcx,
            "qcy": qcy,
            "qemb": qemb,
        })
    return in_maps


def kernel(embeddings, coordinates, anchor_idx):
    global LAST_RESULTS
    in_maps = make_in_maps(embeddings, coordinates, anchor_idx)
    nc = _build_program()
    kres = run_bass_kernel_spmd(nc, in_maps, list(range(NCORES)), trace=TRACE)
    LAST_RESULTS = kres
    tl_sum = np.float32(0.0)
    cnt = np.float32(0.0)
    for k in range(NCORES):
        out = kres.results[k]
        tl_sum += np.asarray(out["tlm"], dtype=np.float32).sum(dtype=np.float32)
        cnt += np.asarray(out["vld"], dtype=np.float32).sum(dtype=np.float32)
    loss = tl_sum / max(cnt, np.float32(1.0))
    return np.asarray(loss, dtype=np.float32)


# revision 27
# speedup vs baseline: 1.3436x; 1.3436x over previous
"""Trainium2 Bass kernel for CustomTripletMarginLoss (retrieval_knn).

Strategy (matches the sharding hint): shard the 8192 anchors across the 8
NeuronCores (1024 anchors each). Anchors are packed [128 partitions x 8
tiles] per core.

v2: the negated squared coordinate distances
    nd2[p, j] = -((cx[j] - ax[p])^2 + (cy[j] - ay[p])^2)
are produced by the (otherwise idle) TensorEngine via the quadratic
expansion as a K=4 matmul:
    nd2 = 2*ax*cx + 2*ay*cy - an2*1 - 1*cn2
with lhsT = [2ax; 2ay; -an2; -1] (stationary, [4, 128] per tile) and
rhs = [cx; cy; 1; cn2] (moving, [4, 8192]). an2 is host-gathered from the
same cn2 array so the self column sums to exactly 0.0 under fp32 PSUM
accumulation; self therefore stays the row max of nd2 (slot 0 of max8) and
no explicit self-masking is needed. ACT evicts PSUM quarters to SBUF; DVE
does the same one-sweep mining as v1 (row-min, max8, find_index8) but no
longer spends passes producing nd2 (which was the v1 bottleneck: DVE was
85% busy with 4 full sweeps per tile).

Mining works in negated squared-distance space (sqrt is monotone;
d < 5 <=> d2 < 25). A row is valid iff min nd2 <= -25 (a negative exists)
and the nearest non-self neighbor (max8 slot 1) has nd2 > -25 (a positive
exists). Embedding rows for pos/neg are gathered with indirect DMA, the
triplet loss terms are computed on-chip, and each core writes per-anchor
masked losses + valid flags; the host sums and divides.

v3: the neg side (argmax of distance) is exact over the convex hull of
the 2D coordinates: ||a - x|| is convex in x, so its max over the point
set is attained at a hull vertex. The host computes the hull (10 vertices
for this data; padded to 16, sorted ascending so find_index8's
first-occurrence matches jnp.argmax tie-breaking), and the device mines
neg over just those 16 candidates with exact ACT (q - a)^2 arithmetic.
This removes the full-row TENSOR_REDUCE(min) sweep - 1 of the 3 DVE
sweeps per tile - and the neg embedding is gathered from a host-staged
16-row candidate table.
"""

import numpy as np

import concourse.bacc as bacc
import concourse.bass as bass
import concourse.mybir as mybir
from concourse.bass import IndirectOffsetOnAxis
from concourse.bass_utils import run_bass_kernel_spmd
from concourse.tile import TileContext

N = 8192          # samples / anchors
D = 512           # embedding dim
NCORES = 8
P = 128           # partitions
PA = N // NCORES  # anchors per core
T = PA // P       # row-tiles per core
MARGIN = 0.1
NTHRESH = -25.0   # negated squared mining threshold: (MARGIN*100/2)^2 = 25
QC = 2048         # psum quarter-tile columns (4 banks)
MM = 512          # max moving free dim per matmul
H = 16            # neg-side hull candidate slots (adaptive: grown in
                  # kernel() if the hull is larger; >= 8 for find_index8)

F32 = mybir.dt.float32
I32 = mybir.dt.int32
U32 = mybir.dt.uint32

# test.py hooks: set TRACE=True before calling kernel() to capture an NTFF
# profile; the raw BassKernelResults land in LAST_RESULTS.
TRACE = False
LAST_RESULTS = None


def _build_program() -> bass.Bass:
    Act = mybir.ActivationFunctionType
    Alu = mybir.AluOpType

    nc = bacc.Bacc()
    emb = nc.declare_dram_parameter("emb", [N, D], F32, isOutput=False)
    F32R = mybir.dt.float32r
    rhs = nc.declare_dram_parameter("rhs", [4, N], F32R, isOutput=False)
    lt = nc.declare_dram_parameter("lt", [4, PA], F32R, isOutput=False)
    nax = nc.declare_dram_parameter("nax", [P, T], F32, isOutput=False)
    nay = nc.declare_dram_parameter("nay", [P, T], F32, isOutput=False)
    # anchor embeddings, host pre-gathered (anchor_idx is host-known), laid
    # out so tile t is the column block [:, t*D:(t+1)*D]
    ae = nc.declare_dram_parameter("ae", [P, T * D], F32, isOutput=False)
    # hull candidates: coords broadcast to all partitions + embedding table
    qcx = nc.declare_dram_parameter("qcx", [P, H], F32, isOutput=False)
    qcy = nc.declare_dram_parameter("qcy", [P, H], F32, isOutput=False)
    qemb = nc.declare_dram_parameter("qemb", [H, D], F32, isOutput=False)
    o_tlm = nc.declare_dram_parameter("tlm", [P, T], F32, isOutput=True)
    o_vld = nc.declare_dram_parameter("vld", [P, T], F32, isOutput=True)
    # debug: per tile t, col t*8+0 = pos_idx, col t*8+7 = neg_idx
    o_idx = nc.declare_dram_parameter("idx", [P, T * 8], U32, isOutput=True)

    with TileContext(nc) as tc:
        with (
            tc.tile_pool(name="const", bufs=1) as pc,
            tc.tile_pool(name="nd", bufs=2) as pn,
            tc.tile_pool(name="psum", bufs=2, space="PSUM") as pp,
            tc.tile_pool(name="small", bufs=3) as ps,
            tc.tile_pool(name="embt", bufs=2) as pe,
        ):
            rhs_t = pc.tile_from(rhs[:], name="rhs_t")
            lt_t = pc.tile_from(lt[:], name="lt_t")
            nax_t = pc.tile_from(nax[:], name="nax_t")
            nay_t = pc.tile_from(nay[:], name="nay_t")
            qcx_t = pc.tile_from(qcx[:], name="qcx_t")
            qcy_t = pc.tile_from(qcy[:], name="qcy_t")
            zero_t = pc.tile([P, 1], F32, name="zero_t")
            margin_t = pc.tile([P, 1], F32, name="margin_t")
            nc.vector.memset(zero_t[:], 0.0)
            nc.vector.memset(margin_t[:], MARGIN)
            tlm_acc = pc.tile([P, T], F32, name="tlm_acc")
            vld_acc = pc.tile([P, T], F32, name="vld_acc")
            idx_acc = pc.tile([P, T * 8], U32, name="idx_acc")

            def phase2(st):
                # triplet-loss tail for a mined tile; DVE subs run after the
                # NEXT tile's mining start so the indirect gathers have
                # landed. float32r matmul noise (~5e-5 abs) can swap which of
                # max8's slots 0/1 is self vs the nearest neighbor, so both
                # candidate embeddings were gathered; self is identified by
                # its bit-exact-zero embedding distance to the anchor and the
                # pos leg is blended arithmetically (pos = slot0 unless
                # slot0 is self).
                t = st["t"]
                dp0 = pe.tile([P, D], F32, name="dp0")
                dp1 = pe.tile([P, D], F32, name="dp1")
                dn = pe.tile([P, D], F32, name="dn")
                nc.vector.tensor_sub(dp0, st["ae_g"], st["e0_g"])
                nc.vector.tensor_sub(dp1, st["ae_g"], st["e1_g"])
                nc.vector.tensor_sub(dn, st["ae_g"], st["ne_g"])
                sq0 = pe.tile([P, D], F32, name="sq0")
                sq1 = pe.tile([P, D], F32, name="sq1")
                sqn = pe.tile([P, D], F32, name="sqn")
                pd20 = ps.tile([P, 1], F32, name="pd20")
                pd21 = ps.tile([P, 1], F32, name="pd21")
                nd2e = ps.tile([P, 1], F32, name="nd2e")
                nc.scalar.activation(sq0, dp0, Act.Square, bias=zero_t[:],
                                     accum_out=pd20)
                nc.scalar.activation(sq1, dp1, Act.Square, bias=zero_t[:],
                                     accum_out=pd21)
                nc.scalar.activation(sqn, dn, Act.Square, bias=zero_t[:],
                                     accum_out=nd2e)
                take0 = ps.tile([P, 1], F32, name="take0")
                nc.vector.tensor_scalar(out=take0, in0=pd20, scalar1=0.0,
                                        scalar2=None, op0=Alu.not_equal)
                posd0 = ps.tile([P, 1], F32, name="posd0")
                posd1 = ps.tile([P, 1], F32, name="posd1")
                negd = ps.tile([P, 1], F32, name="negd")
                nc.scalar.activation(posd0, pd20, Act.Sqrt, bias=zero_t[:])
                nc.scalar.activation(posd1, pd21, Act.Sqrt, bias=zero_t[:])
                nc.scalar.activation(negd, nd2e, Act.Sqrt, bias=zero_t[:])
                # posd = posd1 + take0 * (posd0 - posd1)
                dd = ps.tile([P, 1], F32, name="dd")
                nc.vector.tensor_sub(dd, posd0, posd1)
                t0d = ps.tile([P, 1], F32, name="t0d")
                nc.vector.tensor_mul(t0d, take0, dd)
                posd = ps.tile([P, 1], F32, name="posd")
                nc.vector.tensor_add(posd, t0d, posd1)
                # validity-pos on the chosen slot's coordinate value
                g0 = ps.tile([P, 1], F32, name="g0")
                g1 = ps.tile([P, 1], F32, name="g1")
                nc.vector.tensor_scalar(out=g0, in0=st["mr"][:, 0:1],
                                        scalar1=NTHRESH, scalar2=None,
                                        op0=Alu.is_gt)
                nc.vector.tensor_scalar(out=g1, in0=st["mr"][:, 1:2],
                                        scalar1=NTHRESH, scalar2=None,
                                        op0=Alu.is_gt)
                gd = ps.tile([P, 1], F32, name="gd")
                nc.vector.tensor_sub(gd, g0, g1)
                tg = ps.tile([P, 1], F32, name="tg")
                nc.vector.tensor_mul(tg, take0, gd)
                v2 = ps.tile([P, 1], F32, name="v2")
                nc.vector.tensor_add(v2, tg, g1)
                pmn = ps.tile([P, 1], F32, name="pmn")
                nc.vector.tensor_sub(pmn, posd, negd)
                tl = ps.tile([P, 1], F32, name="tl")
                nc.scalar.activation(tl, pmn, Act.Relu, bias=margin_t[:],
                                     scale=1.0)
                nc.vector.scalar_tensor_tensor(
                    out=vld_acc[:, t:t + 1], in0=st["mr"][:, 8:9],
                    scalar=NTHRESH, in1=v2, op0=Alu.is_le, op1=Alu.mult)
                nc.vector.tensor_mul(tlm_acc[:, t:t + 1], tl,
                                     vld_acc[:, t:t + 1])

            pending = None
            for t in range(T):
                nd2 = pn.tile([P, N], F32, name="nd2")
                for h in range(N // QC):
                    pt = pp.tile([P, QC], F32, name="pt")
                    for c in range(QC // MM):
                        j0 = h * QC + c * MM
                        nc.tensor.matmul(
                            pt[:, c * MM:(c + 1) * MM],
                            lt_t[:, t * P:(t + 1) * P],
                            rhs_t[:, j0:j0 + MM],
                            start=True, stop=True)
                    nc.scalar.activation(nd2[:, h * QC:(h + 1) * QC], pt[:],
                                         Act.Copy, bias=0.0, scale=1.0)

                # neg side: exact (q - a)^2 over the H hull candidates;
                # mr packs [max8 | cand-negmin] so mr[:, 1:2] (nn value) and
                # mr[:, 8:9] (global min of nd2, via the hull) feed the same
                # validity checks as before
                mr = ps.tile([P, 9], F32, name="mr")
                uq = ps.tile([P, H], F32, name="uq")
                vq = ps.tile([P, H], F32, name="vq")
                nq = ps.tile([P, H], F32, name="nq")
                nc.scalar.activation(uq, qcx_t[:], Act.Square,
                                     bias=nax_t[:, t:t + 1], scale=1.0)
                nc.scalar.activation(vq, qcy_t[:], Act.Square,
                                     bias=nay_t[:, t:t + 1], scale=1.0)
                nc.vector.scalar_tensor_tensor(
                    out=nq, in0=uq, scalar=-1.0, in1=vq,
                    op0=Alu.mult, op1=Alu.subtract)
                nc.vector.tensor_reduce(
                    out=mr[:, 8:9], in_=nq, axis=mybir.AxisListType.X,
                    op=Alu.min)
                cidx = ps.tile([P, 8], U32, name="cidx")
                nc.vector.max_index(out=cidx,
                                    in_max=mr[:, 8:9].broadcast_to([P, 8]),
                                    in_values=nq)
                ne_g = pe.tile([P, D], F32, name="ne_g")
                nc.gpsimd.indirect_dma_start(
                    out=ne_g, out_offset=None, in_=qemb[:],
                    in_offset=IndirectOffsetOnAxis(
                        ap=cidx[:, 0:1], axis=0))
                if pending is not None:
                    # previous tile's loss tail lands here, two DVE passes
                    # after its gathers were issued, so they have surely
                    # completed and the subs don't stall
                    phase2(pending)
                nc.vector.max(out=mr[:, 0:8], in_=nd2)
                # probes are max8's own output, so every probe is present in
                # the row; slot 1 is the nearest non-self neighbor (slot 0 is
                # the anchor itself at exactly 0.0)
                nc.vector.max_index(out=idx_acc[:, t * 8:(t + 1) * 8],
                                    in_max=mr[:, 0:8], in_values=nd2)

                ae_g = pe.tile([P, D], F32, name="ae_g")
                e0_g = pe.tile([P, D], F32, name="e0_g")
                e1_g = pe.tile([P, D], F32, name="e1_g")
                nc.sync.dma_start(out=ae_g, in_=ae[:, t * D:(t + 1) * D])
                nc.gpsimd.indirect_dma_start(
                    out=e0_g, out_offset=None, in_=emb[:],
                    in_offset=IndirectOffsetOnAxis(
                        ap=idx_acc[:, t * 8:t * 8 + 1], axis=0))
                nc.gpsimd.indirect_dma_start(
                    out=e1_g, out_offset=None, in_=emb[:],
                    in_offset=IndirectOffsetOnAxis(
                        ap=idx_acc[:, t * 8 + 1:t * 8 + 2], axis=0))
                pending = {"t": t, "ae_g": ae_g, "e0_g": e0_g, "e1_g": e1_g,
                           "ne_g": ne_g, "mr": mr}
            phase2(pending)

            nc.sync.dma_start(out=o_tlm[:], in_=tlm_acc[:])
            nc.sync.dma_start(out=o_vld[:], in_=vld_acc[:])
            nc.sync.dma_start(out=o_idx[:], in_=idx_acc[:])
    # Bacc.finalize runs compile() (event-semaphore legalization for
    # multi-wait instructions, register allocation, DCE). The PJRT run
    # path does not call it for prebuilt modules.
    nc.finalize()
    return nc


def _hull_indices(cx, cy):
    """Andrew monotone chain; returns sorted original indices of the hull
    vertices of the (cx, cy) point set."""
    pts = np.stack([cx, cy], axis=1).astype(np.float64)
    order = np.lexsort((pts[:, 1], pts[:, 0]))
    p = pts[order]

    def half(rng):
        st = []
        for i in rng:
            while len(st) >= 2:
                o, a = p[st[-2]], p[st[-1]]
                if (a[0] - o[0]) * (p[i][1] - o[1]) - \
                   (a[1] - o[1]) * (p[i][0] - o[0]) <= 0:
                    st.pop()
                else:
                    break
            st.append(i)
        return st

    lower = half(range(len(p)))
    upper = half(range(len(p) - 1, -1, -1))
    return np.unique(order[np.array(lower[:-1] + upper[:-1])])


def make_in_maps(embeddings, coordinates, anchor_idx):
    emb = np.ascontiguousarray(np.asarray(embeddings, dtype=np.float32))
    coord = np.asarray(coordinates, dtype=np.float32)
    ai = np.asarray(anchor_idx).astype(np.int64)
    cx, cy = coord[:, 0].copy(), coord[:, 1].copy()
    cn2 = cx * cx + cy * cy  # fp32; an2 below indexes this same array so
    # the self column of the matmul sums to exactly zero
    rhs = np.ascontiguousarray(
        np.stack([cx, cy, np.ones(N, np.float32), cn2]).astype(np.float32))
    global H
    hv = _hull_indices(cx, cy)
    if len(hv) > H:
        H = int(-(-len(hv) // 8) * 8)
    hv = np.concatenate([hv, np.full(H - len(hv), hv[0], dtype=hv.dtype)])
    qcx = np.ascontiguousarray(
        np.broadcast_to(cx[hv], (P, H)).astype(np.float32))
    qcy = np.ascontiguousarray(
        np.broadcast_to(cy[hv], (P, H)).astype(np.float32))
    qemb = np.ascontiguousarray(emb[hv])
    in_maps = []
    for k in range(NCORES):
        sl = ai[k * PA:(k + 1) * PA]
        ae_core = emb[sl].reshape(T, P, D).transpose(1, 0, 2).reshape(P, T * D)
        lt = np.ascontiguousarray(np.stack([
            2.0 * cx[sl], 2.0 * cy[sl], -cn2[sl],
            -np.ones(PA, np.float32)]).astype(np.float32))
        in_maps.append({
            "emb": emb,
            "rhs": rhs,
            "lt": lt,
            "nax": np.ascontiguousarray((-cx[sl]).reshape(T, P).T),
            "nay": np.ascontiguousarray((-cy[sl]).reshape(T, P).T),
            "ae": np.ascontiguousarray(ae_core),
            "qcx": qcx,
            "qcy": qcy,
            "qemb": qemb,
        })
    return in_maps


def kernel(embeddings, coordinates, anchor_idx):
    global LAST_RESULTS
    in_maps = make_in_maps(embeddings, coordinates, anchor_idx)
    nc = _build_program()
    kres = run_bass_kernel_spmd(nc, in_maps, list(range(NCORES)), trace=TRACE)
    LAST_RESULTS = kres
    tl_sum = np.float32(0.0)
    cnt = np.float32(0.0)
    for k in range(NCORES):
        out = kres.results[k]
        tl_sum += np.asarray(out["tlm"], dtype=np.float32).sum(dtype=np.float32)
        cnt += np.asarray(out["vld"], dtype=np.float32).sum(dtype=np.float32)
    loss = tl_sum / max(cnt, np.float32(1.0))
    return np.asarray(loss, dtype=np.float32)
